# revision 1
# baseline (speedup 1.0000x reference)
"""Trainium2 Bass kernel for nn_DepthwiseConvOverTimeLayer.

Pipeline (per core, C-sharded 8 ways):
  x --(depthwise 3x3 conv as per-channel banded matmul on PE, K=50 incl bias row)-->
  y in PSUM [196=(hw,m) split 112+84, bt=320] --(DVE reduce_max over t)-->
  ymax [*, ch, b] --(PE transpose per b)--> ymaxT [ch, b, m, hw_pad81]
  --(conv2: 36 block-diag matmuls on PE, 9-tap PSUM accumulation)--> z --(+bias, DMA out)

Channel coupling: conv2 group c2 consumes depthwise channels
c = 256*m2 + c2//4 (m2=0..3) at m = c2 % 4, so core j owns
c2 in [128j, 128j+128) and dw channels {256a + 32j + s : a,s}.
"""

import numpy as np

B, T, H, W, C, M = 16, 20, 7, 7, 1024, 4
KD = KP = 3
NCORES = 8
CL = 128          # dw channels per core
HWQ = 49          # 7*7
BT = B * T        # 320
NGROUPS = 32      # stage-A channel groups of 4
F16 = np.float16

TRACE = False
LAST_RESULTS = None
_NC_CACHE = None


# ----------------------------------------------------------------- host prep
def _core_channels(j):
    l = np.arange(CL)
    return 256 * (l // 32) + 32 * j + (l % 32)


def build_core_inputs(x, dw_w, dw_b, conv_w, conv_b, j):
    """Per-core input arrays (all float16 except bias2)."""
    cj = _core_channels(j)

    # xb [50, 128, 320]: partitions = hw_in (+ ones row 49), free = (ch, bt)
    xs = np.asarray(x, np.float32)[:, :, :, :, cj].reshape(BT, HWQ, CL)
    xb = np.empty((50, CL, BT), np.float32)
    xb[:HWQ] = xs.transpose(1, 2, 0)
    xb[HWQ] = 1.0

    # aw [50, 128, 196]: banded depthwise matrix + bias row.
    # col = hw_out*4 + m ; aw[hw_in, l, col] = dw_w[kh, kw, c(l), m]
    aw = np.zeros((50, CL, 196), np.float32)
    wsel = np.asarray(dw_w, np.float32)[:, :, cj, :]          # [3,3,128,4]
    for ho in range(H):
        for wo in range(W):
            col = (ho * W + wo) * 4
            for kh in range(KD):
                for kw in range(KD):
                    hi, wi = ho + kh - 1, wo + kw - 1
                    if 0 <= hi < H and 0 <= wi < W:
                        aw[hi * W + wi, :, col:col + 4] = wsel[kh, kw]
    bcols = np.asarray(dw_b, np.float32).reshape(C, M)[cj]    # [128, 4]
    aw[HWQ] = np.tile(bcols[:, None, :], (1, HWQ, 1)).reshape(CL, 196)

    # a2 [128, 36, 128]: conv2 block-diag weights.
    # a2[p_in=32*m2+s, t9*4+r, p_out=4*s+mo] = conv_w[c2=128j+4s+r, kh, kw, m2, mo]
    a2 = np.zeros((CL, 36, CL), np.float32)
    cw = np.asarray(conv_w, np.float32)
    s = np.arange(32)
    for t9 in range(9):
        kh, kw = divmod(t9, 3)
        for r in range(4):
            blk = cw[128 * j + 4 * s + r, kh, kw, :, :]       # [32(s), 4(m2), 4(mo)]
            for m2 in range(4):
                a2[32 * m2 + s[:, None], t9 * 4 + r,
                   4 * s[:, None] + np.arange(4)[None, :]] = blk[:, m2, :]

    # bias2 [128, 4]: bias2[4s+mo, r] = conv_b[128j+4s+r, mo]
    cb = np.asarray(conv_b, np.float32)
    bias2 = np.empty((CL, 4), np.float32)
    for r in range(4):
        bias2[(4 * s[:, None] + np.arange(4)[None, :]).ravel(), r] = \
            cb[128 * j + 4 * s + r, :].ravel()

    ident = np.eye(112, dtype=np.float32)

    return {"xb": xb.astype(F16), "aw": aw.astype(F16), "a2": a2.astype(F16),
            "bias2": bias2.astype(np.float32), "ident": ident.astype(F16)}


def assemble_output(core_outs):
    """core_outs[j]['zout'] [128=(4s+mo), 4=r, 16=b, 49=hw] -> full (B,M,H,W,C)."""
    out = np.empty((B, M, H, W, C), np.float32)
    for j in range(NCORES):
        z = np.asarray(core_outs[j]["zout"]).reshape(32, 4, 4, B, HWQ)  # s,mo,r,b,hw
        zz = z.transpose(3, 1, 4, 0, 2).reshape(B, M, H, W, CL)         # b,mo,h,w,(s r)
        out[:, :, :, :, 128 * j:128 * j + 128] = zz
    return out


# ----------------------------------------------------------------- bass build
def build_bass():
    import concourse.mybir as mybir
    from concourse import bacc
    from concourse.tile import TileContext

    dt = mybir.dt
    nc = bacc.Bacc()

    xb_d = nc.dram_tensor("xb", [50, CL, BT], dt.float16, kind="ExternalInput")
    aw_d = nc.dram_tensor("aw", [50, CL, 196], dt.float16, kind="ExternalInput")
    a2_d = nc.dram_tensor("a2", [CL, 36, CL], dt.float16, kind="ExternalInput")
    bias2_d = nc.dram_tensor("bias2", [CL, 4], dt.float32, kind="ExternalInput")
    ident_d = nc.dram_tensor("ident", [112, 112], dt.float16, kind="ExternalInput")
    zout_d = nc.dram_tensor("zout", [CL, 4, B, HWQ], dt.float32,
                            kind="ExternalOutput")

    with TileContext(nc) as tc:
        with tc.tile_pool(name="const", bufs=1) as cpool:
            a2_t = cpool.tile([CL, 36, CL], dt.float16)
            bias2_t = cpool.tile([CL, 4], dt.float32)
            ident_t = cpool.tile([112, 112], dt.float16)
            ymaxA = cpool.tile([112, CL, B], dt.float16)   # [(hw<28,m), ch, b]
            ymaxB = cpool.tile([84, CL, B], dt.float16)    # [(hw>=28,m), ch, b]
            ymaxT = cpool.tile([CL, B, 4, 81], dt.float16)  # [ch, b, m, hw_pad]
            zsb = cpool.tile([CL, 4, B, HWQ], dt.float32)

            nc.gpsimd.memset(ymaxT[:], 0.0)

            # ---------------- stage A: depthwise conv + temporal max
            xts = []
            ats = []
            for q in range(4):
                xt = cpool.tile([50, 32, BT], dt.float16, tag=f"x{q}")
                nc.sync.dma_start(out=xt[:], in_=xb_d[:, 32 * q:32 * q + 32, :])
                at = cpool.tile([50, 32, 196], dt.float16, tag=f"a{q}")
                nc.sync.dma_start(out=at[:], in_=aw_d[:, 32 * q:32 * q + 32, :])
                xts.append(xt)
                ats.append(at)
            nc.sync.dma_start(out=a2_t[:], in_=a2_d[:])
            nc.sync.dma_start(out=bias2_t[:], in_=bias2_d[:])
            nc.sync.dma_start(out=ident_t[:], in_=ident_d[:])
            with tc.tile_pool(name="psA", bufs=1, space="PSUM") as psA, \
                 tc.tile_pool(name="psB", bufs=1, space="PSUM") as psB:
                for g in range(NGROUPS):
                    xg = xts[g // 8]
                    ag = ats[g // 8]
                    co = 4 * (g % 8)

                    pa = psA.tile([112, 4, 512], dt.float32)
                    pb = psB.tile([84, 4, 512], dt.float32)
                    for i in range(4):
                        rhs = xg[:, co + i, :]
                        nc.tensor.matmul(pa[:, i, 0:320],
                                         ag[:, co + i, 0:112], rhs,
                                         start=True, stop=True)
                        nc.tensor.matmul(pb[:, i, 0:320],
                                         ag[:, co + i, 112:196], rhs,
                                         start=True, stop=True)
                    ina = pa[:, :, 0:320].rearrange("p c (b t) -> p c b t", t=T)
                    nc.vector.reduce_max(ymaxA[:, 4 * g:4 * g + 4, :], ina,
                                         axis=mybir.AxisListType.X)
                    inb = pb[:, :, 0:320].rearrange("p c (b t) -> p c b t", t=T)
                    nc.vector.reduce_max(ymaxB[:, 4 * g:4 * g + 4, :], inb,
                                         axis=mybir.AxisListType.X)

                # ------------ stage B + C interleaved by b-half
                for bh in range(2):
                    for b in range(8 * bh, 8 * bh + 8):
                        ta = psA.tile([CL, 112], dt.float16, tag="pa")
                        nc.tensor.transpose(ta[:], ymaxA[:, :, b], ident_t[:])
                        tb = psB.tile([CL, 84], dt.float16, tag="pb")
                        nc.tensor.transpose(tb[:], ymaxB[:, :, b],
                                            ident_t[0:84, 0:84])
                        dsta = ymaxT[:, b].rearrange(
                            "p m (hh ww) -> p m hh ww",
                            hh=9, ww=9)[:, :, 1:5, 1:8]
                        nc.scalar.copy(dsta, ta[:].rearrange(
                            "p (h w m) -> p m h w", h=4, w=7))
                        dstb = ymaxT[:, b].rearrange(
                            "p m (hh ww) -> p m hh ww",
                            hh=9, ww=9)[:, :, 5:8, 1:8]
                        nc.scalar.copy(dstb, tb[:].rearrange(
                            "p (h w m) -> p m h w", h=3, w=7))

                    for r in range(4):
                        pz = psB.tile([CL, 8, HWQ], dt.float32, tag="pb")
                        for t9 in range(9):
                            kh, kw = divmod(t9, 3)
                            rhs = ymaxT[:, 8 * bh:8 * bh + 8, r, :].rearrange(
                                "p b (hh ww) -> p b hh ww", hh=9, ww=9)[
                                :, :, kh:kh + 7, kw:kw + 7]
                            nc.tensor.matmul(pz[:], a2_t[:, 4 * t9 + r, :], rhs,
                                             start=(t9 == 0), stop=(t9 == 8))
                        zslice = zsb[:, r, 8 * bh:8 * bh + 8, :]
                        nc.vector.tensor_scalar_add(zslice, pz[:],
                                                    bias2_t[:, r:r + 1])
                        nc.sync.dma_start(
                            out=zout_d[:, r, 8 * bh:8 * bh + 8, :], in_=zslice)

    nc.finalize()
    return nc


def _get_nc():
    global _NC_CACHE
    if _NC_CACHE is None:
        _NC_CACHE = build_bass()
    return _NC_CACHE


# ----------------------------------------------------------------- entry point
def kernel(x, dw_w, dw_b, conv_w, conv_b):
    global LAST_RESULTS
    from concourse.bass_utils import run_bass_kernel_spmd

    in_maps = [build_core_inputs(x, dw_w, dw_b, conv_w, conv_b, j)
               for j in range(NCORES)]
    nc = _get_nc()
    res = run_bass_kernel_spmd(nc, in_maps, core_ids=list(range(NCORES)),
                               trace=TRACE)
    LAST_RESULTS = res
    return assemble_output(res.results)



# revision 4
# speedup vs baseline: 1.0785x; 1.0785x over previous
"""Trainium2 Bass kernel for nn_DepthwiseConvOverTimeLayer.

Pipeline (per core, C-sharded 8 ways):
  x --(depthwise 3x3 conv as per-channel banded matmul on PE, K=100 incl bias
  row; rows 50-99 are zero-weight padding that keeps >=4 PE row-strips active
  so the HAM clock gate un-throttles to 2.4 GHz)-->
  y in PSUM fp16 [196=(hw,m) split 112+84, bt=320]
  --(112-half: DVE reduce_max 2x direct from PSUM;
     84-half: ACT copy PSUM->SBUF fp16 then DVE reduce_max 4x)-->
  ymax [*, ch, b] --(PE transpose per b)--> ymaxT [ch, b, m, hw_pad81]
  --(conv2: 36 block-diag matmuls on PE, 9-tap PSUM accumulation)--> z
  --(ACT bias-add, DMA out)

Channel coupling: conv2 group c2 consumes depthwise channels
c = 256*m2 + c2//4 (m2=0..3) at m = c2 % 4, so core j owns
c2 in [128j, 128j+128) and dw channels {256a + 32j + s : a,s}.
"""

import numpy as np

B, T, H, W, C, M = 16, 20, 7, 7, 1024, 4
KD = KP = 3
NCORES = 8
CL = 128          # dw channels per core
HWQ = 49          # 7*7
BT = B * T        # 320
NGROUPS = 32      # stage-A channel groups of 4
KPAD = 100        # stage-A contraction rows (50 real + 50 zero-weight pad)
F16 = np.float16

TRACE = False
LAST_RESULTS = None
_NC_CACHE = None


# ----------------------------------------------------------------- host prep
def _core_channels(j):
    l = np.arange(CL)
    return 256 * (l // 32) + 32 * j + (l % 32)


def build_core_inputs(x, dw_w, dw_b, conv_w, conv_b, j):
    """Per-core input arrays (all float16 except bias2)."""
    cj = _core_channels(j)

    # xb [100, 128, 320]: partitions = hw_in (+ ones row 49), rows 50-99
    # replicate rows 0-49 (they are multiplied by zero weights; content just
    # needs to be finite). free = (ch, bt)
    xs = np.asarray(x, np.float32)[:, :, :, :, cj].reshape(BT, HWQ, CL)
    xb = np.empty((KPAD, CL, BT), np.float32)
    xb[:HWQ] = xs.transpose(1, 2, 0)
    xb[HWQ] = 1.0
    xb[50:100] = xb[0:50]

    # aw [100, 128, 196]: banded depthwise matrix + bias row; rows 50-99 = 0.
    # col = hw_out*4 + m ; aw[hw_in, l, col] = dw_w[kh, kw, c(l), m]
    aw = np.zeros((KPAD, CL, 196), np.float32)
    wsel = np.asarray(dw_w, np.float32)[:, :, cj, :]          # [3,3,128,4]
    for ho in range(H):
        for wo in range(W):
            col = (ho * W + wo) * 4
            for kh in range(KD):
                for kw in range(KD):
                    hi, wi = ho + kh - 1, wo + kw - 1
                    if 0 <= hi < H and 0 <= wi < W:
                        aw[hi * W + wi, :, col:col + 4] = wsel[kh, kw]
    bcols = np.asarray(dw_b, np.float32).reshape(C, M)[cj]    # [128, 4]
    aw[HWQ] = np.tile(bcols[:, None, :], (1, HWQ, 1)).reshape(CL, 196)

    # a2 [128, 36, 128]: conv2 block-diag weights.
    # a2[p_in=32*m2+s, t9*4+r, p_out=4*s+mo] = conv_w[c2=128j+4s+r, kh, kw, m2, mo]
    a2 = np.zeros((CL, 36, CL), np.float32)
    cw = np.asarray(conv_w, np.float32)
    s = np.arange(32)
    for t9 in range(9):
        kh, kw = divmod(t9, 3)
        for r in range(4):
            blk = cw[128 * j + 4 * s + r, kh, kw, :, :]       # [32(s), 4(m2), 4(mo)]
            for m2 in range(4):
                a2[32 * m2 + s[:, None], t9 * 4 + r,
                   4 * s[:, None] + np.arange(4)[None, :]] = blk[:, m2, :]

    # bias2 [128, 4]: bias2[4s+mo, r] = conv_b[128j+4s+r, mo]
    cb = np.asarray(conv_b, np.float32)
    bias2 = np.empty((CL, 4), np.float32)
    for r in range(4):
        bias2[(4 * s[:, None] + np.arange(4)[None, :]).ravel(), r] = \
            cb[128 * j + 4 * s + r, :].ravel()

    ident = np.eye(112, dtype=np.float32)

    return {"xb": xb.astype(F16), "aw": aw.astype(F16), "a2": a2.astype(F16),
            "bias2": bias2.astype(np.float32), "ident": ident.astype(F16)}


def assemble_output(core_outs):
    """core_outs[j]['zout'] [128=(4s+mo), 4=r, 16=b, 49=hw] -> full (B,M,H,W,C)."""
    out = np.empty((B, M, H, W, C), np.float32)
    for j in range(NCORES):
        z = np.asarray(core_outs[j]["zout"]).reshape(32, 4, 4, B, HWQ)  # s,mo,r,b,hw
        zz = z.transpose(3, 1, 4, 0, 2).reshape(B, M, H, W, CL)         # b,mo,h,w,(s r)
        out[:, :, :, :, 128 * j:128 * j + 128] = zz
    return out


# ----------------------------------------------------------------- bass build
def build_bass():
    import concourse.mybir as mybir
    from concourse import bacc
    from concourse.tile import TileContext

    dt = mybir.dt
    nc = bacc.Bacc()

    xb_d = nc.dram_tensor("xb", [KPAD, CL, BT], dt.float16, kind="ExternalInput")
    aw_d = nc.dram_tensor("aw", [KPAD, CL, 196], dt.float16, kind="ExternalInput")
    a2_d = nc.dram_tensor("a2", [CL, 36, CL], dt.float16, kind="ExternalInput")
    bias2_d = nc.dram_tensor("bias2", [CL, 4], dt.float32, kind="ExternalInput")
    ident_d = nc.dram_tensor("ident", [112, 112], dt.float16, kind="ExternalInput")
    zout_d = nc.dram_tensor("zout", [CL, 4, B, HWQ], dt.float32,
                            kind="ExternalOutput")

    NCHUNK = 8                 # input DMA chunks (16 channels each)
    CPC = CL // NCHUNK         # channels per chunk

    with TileContext(nc) as tc:
        with tc.tile_pool(name="const", bufs=1) as cpool:
            a2_t = cpool.tile([CL, 36, CL], dt.float16)
            bias2_t = cpool.tile([CL, 4], dt.float32)
            ident_t = cpool.tile([112, 112], dt.float16)
            ymaxA = cpool.tile([112, CL, B], dt.float16)   # [(hw<28,m), ch, b]
            ymaxB = cpool.tile([84, CL, B], dt.float16)    # [(hw>=28,m), ch, b]
            ymaxT = cpool.tile([CL, B, 4, 81], dt.float16)  # [ch, b, m, hw_pad]
            zsb = cpool.tile([CL, 4, B, HWQ], dt.float32)

            nc.gpsimd.memset(ymaxT[:], 0.0)

            # ---------------- input DMAs, chunked for early compute start
            xts = []
            ats = []
            for q in range(NCHUNK):
                xt = cpool.tile([KPAD, CPC, BT], dt.float16, tag=f"x{q}")
                nc.sync.dma_start(out=xt[:],
                                  in_=xb_d[:, CPC * q:CPC * q + CPC, :])
                at = cpool.tile([KPAD, CPC, 196], dt.float16, tag=f"a{q}")
                nc.sync.dma_start(out=at[:],
                                  in_=aw_d[:, CPC * q:CPC * q + CPC, :])
                xts.append(xt)
                ats.append(at)
            # const loads on the ACT hwdge ring (keeps SP ring for the bulk)
            nc.scalar.dma_start(out=a2_t[:], in_=a2_d[:])
            nc.scalar.dma_start(out=bias2_t[:], in_=bias2_d[:])
            nc.scalar.dma_start(out=ident_t[:], in_=ident_d[:])

            # ---------------- stage A: depthwise conv + temporal max
            # 3-channel groups: PSUM tile [*, 3, 320] fp32 = exactly 2 banks;
            # slot 1 straddles the bank boundary so its matmul is split at
            # col 192 (byte 2048). bufs=2 double-buffers PE vs reduces.
            with tc.tile_pool(name="psA", bufs=2, space="PSUM") as psA, \
                 tc.tile_pool(name="psB", bufs=2, space="PSUM") as psB, \
                 tc.tile_pool(name="ysb", bufs=3) as ysbp:
                NG3 = (CL + 2) // 3          # 43 groups of <=3 channels
                for g in range(NG3):
                    c0 = 3 * g
                    nch = min(3, CL - c0)

                    pa = psA.tile([112, 3, 320], dt.float32)
                    pb = psB.tile([84, 3, 320], dt.float32)
                    for i in range(nch):
                        c = c0 + i
                        xg = xts[c // CPC]
                        ag = ats[c // CPC]
                        ci = c % CPC
                        rhs = xg[:, ci, :]
                        la = ag[:, ci, 0:112]
                        lb = ag[:, ci, 112:196]
                        if i == 1:
                            # split at the PSUM bank boundary
                            nc.tensor.matmul(pa[:, 1, 0:192], la, rhs[:, 0:192],
                                             start=True, stop=True)
                            nc.tensor.matmul(pa[:, 1, 192:320], la,
                                             rhs[:, 192:320],
                                             start=True, stop=True)
                            nc.tensor.matmul(pb[:, 1, 0:192], lb, rhs[:, 0:192],
                                             start=True, stop=True)
                            nc.tensor.matmul(pb[:, 1, 192:320], lb,
                                             rhs[:, 192:320],
                                             start=True, stop=True)
                        else:
                            nc.tensor.matmul(pa[:, i, 0:320], la, rhs,
                                             start=True, stop=True)
                            nc.tensor.matmul(pb[:, i, 0:320], lb, rhs,
                                             start=True, stop=True)

                    # A-half (112): mostly DVE-direct reduce from fp32 PSUM;
                    # a subset detours via ACT copy to balance engine loads
                    ina = pa[:, 0:nch, :].rearrange("p c (b t) -> p c b t", t=T)
                    outa = ymaxA[:, c0:c0 + nch, :]
                    if (g % 4 == 1) or (g == NG3 - 1):
                        ysa = ysbp.tile([112, 3, 320], dt.float16, tag="ysa")
                        nc.scalar.copy(ysa[:, 0:nch, :], pa[:, 0:nch, :])
                        nc.vector.reduce_max(
                            outa, ysa[:, 0:nch, :].rearrange(
                                "p c (b t) -> p c b t", t=T),
                            axis=mybir.AxisListType.X)
                    else:
                        nc.vector.reduce_max(outa, ina,
                                             axis=mybir.AxisListType.X)
                    # B-half (84): ACT copy PSUM->SBUF fp16, then DVE 4x reduce
                    ysb_t = ysbp.tile([84, 3, 320], dt.float16, tag="ysb")
                    nc.scalar.copy(ysb_t[:, 0:nch, :], pb[:, 0:nch, :])
                    inb = ysb_t[:, 0:nch, :].rearrange(
                        "p c (b t) -> p c b t", t=T)
                    outb = ymaxB[:, c0:c0 + nch, :]
                    nc.vector.reduce_max(outb, inb,
                                         axis=mybir.AxisListType.X)

            # ------------ stage B + C interleaved by b-half
            with tc.tile_pool(name="psTa", bufs=2, space="PSUM") as psTa, \
                 tc.tile_pool(name="psTb", bufs=2, space="PSUM") as psTb, \
                 tc.tile_pool(name="psZ", bufs=4, space="PSUM") as psZ:
                for bh in range(2):
                    for b in range(8 * bh, 8 * bh + 8):
                        ta = psTa.tile([CL, 112], dt.float16, tag="ta")
                        nc.tensor.transpose(ta[:], ymaxA[:, :, b], ident_t[:])
                        tb = psTb.tile([CL, 84], dt.float16, tag="tb")
                        nc.tensor.transpose(tb[:], ymaxB[:, :, b],
                                            ident_t[0:84, 0:84])
                        dsta = ymaxT[:, b].rearrange(
                            "p m (hh ww) -> p m hh ww",
                            hh=9, ww=9)[:, :, 1:5, 1:8]
                        nc.vector.tensor_copy(dsta, ta[:].rearrange(
                            "p (h w m) -> p m h w", h=4, w=7))
                        dstb = ymaxT[:, b].rearrange(
                            "p m (hh ww) -> p m hh ww",
                            hh=9, ww=9)[:, :, 5:8, 1:8]
                        nc.scalar.copy(dstb, tb[:].rearrange(
                            "p (h w m) -> p m h w", h=3, w=7))

                    for r in range(4):
                        pz = psZ.tile([CL, 8, HWQ], dt.float32, tag="pz")
                        for t9 in range(9):
                            kh, kw = divmod(t9, 3)
                            rhs = ymaxT[:, 8 * bh:8 * bh + 8, r, :].rearrange(
                                "p b (hh ww) -> p b hh ww", hh=9, ww=9)[
                                :, :, kh:kh + 7, kw:kw + 7]
                            nc.tensor.matmul(pz[:], a2_t[:, 4 * t9 + r, :], rhs,
                                             start=(t9 == 0), stop=(t9 == 8))
                        zslice = zsb[:, r, 8 * bh:8 * bh + 8, :]
                        nc.scalar.activation(
                            zslice, pz[:],
                            mybir.ActivationFunctionType.Identity,
                            bias=bias2_t[:, r:r + 1])
                        nc.sync.dma_start(
                            out=zout_d[:, r, 8 * bh:8 * bh + 8, :], in_=zslice)

    nc.finalize()
    return nc


def _get_nc():
    global _NC_CACHE
    if _NC_CACHE is None:
        _NC_CACHE = build_bass()
    return _NC_CACHE


# ----------------------------------------------------------------- entry point
def kernel(x, dw_w, dw_b, conv_w, conv_b):
    global LAST_RESULTS
    from concourse.bass_utils import run_bass_kernel_spmd

    in_maps = [build_core_inputs(x, dw_w, dw_b, conv_w, conv_b, j)
               for j in range(NCORES)]
    nc = _get_nc()
    res = run_bass_kernel_spmd(nc, in_maps, core_ids=list(range(NCORES)),
                               trace=TRACE)
    LAST_RESULTS = res
    return assemble_output(res.results)
